# revision 26
# baseline (speedup 1.0000x reference)
"""Trainium2 Bass kernel for nn_Attn (Bahdanau-style attention scores).

Computation (per batch b of B=128):
    energy = tanh(enc[b] @ We.T + (hidden @ Wh.T)[b] + bias)   # (L, H)
    scores = energy @ v                                        # (L,)
    out[b] = softmax(scores)                                   # (1, L)

Sharding: batch data-parallel over 8 NeuronCores (16 batches/core);
weights replicated. Per core the dominant matmul runs in the [h, l]
orientation (contract over d=576) so the (hidden@Wh.T + bias) term
fuses into the tanh as a per-partition activation bias.

Precision split on the d-contraction (the PE column count is the
bottleneck; fp8 DoubleRow streams 2 k-rows/cycle, bf16 streams 1):
  - d 0..255   : fp8 e4m3 via one DoubleRow matmul  (256 k-rows, 2x rate)
  - d 256..639 : bf16 via 3 plain k-tiles           (of which 576+ is pad)
Host-side the fp8/bf16 operands are pre-scaled by 16 (enc) and 256 (We)
to dodge e4m3's tiny subnormal range; the tanh activation un-scales via
its `scale` immediate. Simulated end-to-end rel err: 1.3e-2 (fp8-only
would be 2.4e-2, over the 2e-2 gate; bf16-only 2.5e-3 but 0 PE win).

The N(free)=1024 tiles (both l-halves in one matmul/activation, legal
for 8/16-bit moving operands; psum tiles span 2 banks) halve the
instruction count on PE/ACT/DVE versus N=512.

Stage-2 (scores = v . energy) runs on the DVE as a per-partition
multiply-accumulate chain in bf16 (2x DVE rate), collapsed to scores by
a single K=128 ones-matmul per l-half; the last 2 batches instead use
direct PE v-matmuls so the kernel tail isn't gated on the DVE chain.

Scores are assembled batch-major so softmax runs once over all local
batches at the end.
"""

import numpy as np
import ml_dtypes

import concourse.bacc as bacc
import concourse.mybir as mybir
import concourse.tile as tile
from concourse import bass_utils
from concourse.mybir import ActivationFunctionType as AF
from concourse.mybir import AluOpType, AxisListType

N_CORES = 8
B, L, H = 128, 1024, 512
ONEHOT = 64
DE = H + ONEHOT          # 576, true contraction dim of the big matmul
BL = B // N_CORES        # 16 batches per core
F32 = mybir.dt.float32
BF16 = mybir.dt.bfloat16
F8 = mybir.dt.float8e4

K8 = 256                 # fp8 DoubleRow k-rows (d 0..255)
NBT = 3                  # bf16 k-tiles (d 256..639; 576..639 zero pad)
S_E = 16.0               # host-side enc scale before quantization
S_W = 256.0              # host-side We scale before quantization
INV_S = 1.0 / (S_E * S_W)
NF = 1024                # matmul moving free dim (both l-halves at once)

DR = mybir.MatmulPerfMode.DoubleRow


def build(reps: int = 1, nwarm: int = 40):
    """Build + trace the per-core Bass program. Returns the compiled nc."""
    nc = bacc.Bacc(
        "TRN2", target_bir_lowering=False, debug=False, num_devices=N_CORES
    )
    enc8 = nc.dram_tensor("enc8", [BL, 128, 2, 1024], F8, kind="ExternalInput").ap()
    encb = nc.dram_tensor("encb", [BL, 128, NBT, 1024], BF16, kind="ExternalInput").ap()
    hid = nc.dram_tensor("hid", [H, BL], BF16, kind="ExternalInput").ap()
    wet8 = nc.dram_tensor("wet8", [128, 2, 512], F8, kind="ExternalInput").ap()
    wbt = nc.dram_tensor("wbt", [128, NBT, 512], BF16, kind="ExternalInput").ap()
    wht = nc.dram_tensor("wht", [H, H], BF16, kind="ExternalInput").ap()
    bcol = nc.dram_tensor("bcol", [128, 4], F32, kind="ExternalInput").ap()
    vcol = nc.dram_tensor("vcol", [128, 4], F32, kind="ExternalInput").ap()
    # mask16[:, 15] = ones, else 0; sliced [15-b : 31-b] it puts the ones
    # column at row b of a [16, N] matmul output (batch-major psum scores).
    mask16 = nc.dram_tensor("mask16", [128, 31], BF16, kind="ExternalInput").ap()
    # vmask16[:, ht, 15] = v[ht*128+p], else 0 (direct PE v-contraction path)
    vmask16 = nc.dram_tensor("vmask16", [128, 4, 31], BF16, kind="ExternalInput").ap()
    out = nc.dram_tensor("out", [BL, L], F32, kind="ExternalOutput").ap()

    with tile.TileContext(nc) as tc:
        with (
            tc.tile_pool(name="const", bufs=1) as cpool,
            tc.tile_pool(name="encp", bufs=4) as epool,
            tc.tile_pool(name="energy", bufs=8) as gpool,
            tc.tile_pool(name="cb", bufs=2) as cbpool,
            tc.tile_pool(name="soft", bufs=1) as spool,
            tc.tile_pool(name="stage", bufs=4) as stpool,
            tc.tile_pool(name="ps1", bufs=3, space="PSUM") as ps1,
            tc.tile_pool(name="ps3", bufs=1, space="PSUM") as ps3,
        ):
            # ---- replicated constants ----
            wet8_sb = cpool.tile([128, 2, 512], F8, tag="wet8", name="wet8_sb")
            nc.sync.dma_start(wet8_sb[:], wet8[:, :, :])
            wbt_sb = cpool.tile([128, NBT, 512], BF16, tag="wbt", name="wbt_sb")
            nc.sync.dma_start(wbt_sb[:], wbt[:, :, :])
            wht_sb = []
            for kt in range(4):
                t = cpool.tile([128, H], BF16, tag=f"wht{kt}", name=f"wht{kt}")
                nc.sync.dma_start(t[:], wht[kt * 128 : (kt + 1) * 128, :])
                wht_sb.append(t)
            hid_sb = []
            for kt in range(4):
                t = cpool.tile([128, BL], BF16, tag=f"hid{kt}", name=f"hid{kt}")
                nc.sync.dma_start(t[:], hid[kt * 128 : (kt + 1) * 128, :])
                hid_sb.append(t)
            bcol_sb = cpool.tile([128, 4], F32, tag="bcol", name="bcol_sb")
            nc.sync.dma_start(bcol_sb[:], bcol[:, :])
            vcol_sb = cpool.tile([128, 4], F32, tag="vcol", name="vcol_sb")
            nc.sync.dma_start(vcol_sb[:], vcol[:, :])
            mask16_sb = cpool.tile([128, 31], BF16, tag="mask16", name="mask16_sb")
            nc.sync.dma_start(mask16_sb[:], mask16[:, :])
            vmask16_sb = cpool.tile(
                [128, 4, 31], BF16, tag="vmask16", name="vmask16_sb"
            )
            nc.sync.dma_start(vmask16_sb[:], vmask16[:, :, :])

            for _rep in range(reps):
                # ---- PE warmup: junk DoubleRow matmuls as soon as wet8
                # lands, so the HAM clock-gate reaches 8/8 before real work
                # and stays there over the initial enc DMA window.
                warm = ps1.tile([128, NF], F32, tag="ps1", name="warm")
                for w in range(nwarm):
                    nc.tensor.matmul(
                        warm[:, 0:512],
                        lhsT=wet8_sb[:, :, 0:128],
                        rhs=wet8_sb[:, :, :],
                        start=(w == 0),
                        stop=(w == nwarm - 1),
                        perf_mode=DR,
                    )

                # ---- c[h, b] = (hidden @ Wh.T).T + bias, per-partition h ----
                cb_sb = []
                for ht in range(4):
                    pc = ps1.tile([128, NF], F32, tag="ps1", name=f"pc{ht}")
                    for kt in range(4):
                        nc.tensor.matmul(
                            pc[:, :BL],
                            lhsT=wht_sb[kt][:, ht * 128 : (ht + 1) * 128],
                            rhs=hid_sb[kt][:],
                            start=(kt == 0),
                            stop=(kt == 3),
                        )
                    cbt = cbpool.tile([128, BL], F32, tag=f"cb{ht}", name=f"cb{ht}")
                    nc.vector.tensor_scalar_add(
                        cbt[:], pc[:, :BL], bcol_sb[:, ht : ht + 1]
                    )
                    cb_sb.append(cbt)

                # batch-major psum scores: every batch's stage-2 matmul
                # accumulates into its own row via the shifted mask16 column
                ps_sc = ps3.tile([BL, L], F32, tag="pss", name="ps_sc")

                # ---- main loop over local batch pairs: both batches of a
                # pair stream through each loaded weight tile (4 matmuls
                # per LDWEIGHTS instead of 2) ----
                for bp in range(BL // 2):
                    pair = (2 * bp, 2 * bp + 1)
                    e8t, ebt = [], []
                    for b in pair:
                        t8 = epool.tile([128, 2, 1024], F8, tag="e8", name=f"e8_{b}")
                        nc.sync.dma_start(t8[:], enc8[b])
                        e8t.append(t8)
                        tb = epool.tile(
                            [128, NBT, 1024], BF16, tag="eb", name=f"eb_{b}"
                        )
                        nc.sync.dma_start(tb[:], encb[b])
                        ebt.append(tb)

                    ens = [[], []]
                    for ht in range(4):
                        pe_t = [
                            ps1.tile([128, NF], F32, tag="ps1", name=f"pe{b}_{ht}")
                            for b in pair
                        ]
                        # (ISA caps a matmul's output at 512 elements = 1
                        # psum bank, hence the lh split.)
                        for j in range(2):
                            for lh in range(2):
                                nc.tensor.matmul(
                                    pe_t[j][:, lh * 512 : (lh + 1) * 512],
                                    lhsT=wet8_sb[:, :, ht * 128 : (ht + 1) * 128],
                                    rhs=e8t[j][:, :, lh * 512 : (lh + 1) * 512],
                                    start=True,
                                    stop=False,
                                    perf_mode=DR,
                                )
                        for kt in range(NBT):
                            for j in range(2):
                                for lh in range(2):
                                    nc.tensor.matmul(
                                        pe_t[j][:, lh * 512 : (lh + 1) * 512],
                                        lhsT=wbt_sb[:, kt, ht * 128 : (ht + 1) * 128],
                                        rhs=ebt[j][:, kt, lh * 512 : (lh + 1) * 512],
                                        start=False,
                                        stop=(kt == NBT - 1),
                                    )
                        for j, b in enumerate(pair):
                            en_t = gpool.tile(
                                [128, NF], BF16, tag="en", name=f"en{b}_{ht}",
                                bufs=12,
                            )
                            nc.scalar.activation(
                                en_t[:], pe_t[j][:], AF.Tanh,
                                bias=cb_sb[ht][:, b : b + 1], scale=INV_S,
                            )
                            ens[j].append(en_t)

                    for j, b in enumerate(pair):
                        msl = slice(15 - b, 31 - b)
                        if b < BL - 1:
                            # z[p, l] = sum_ht v_ht[p]*en_ht[p, l]  (DVE, bf16)
                            z = None
                            for ht in range(4):
                                zn = stpool.tile(
                                    [128, NF], BF16, tag="z", name=f"z{b}_{ht}",
                                    bufs=8,
                                )
                                if z is None:
                                    nc.vector.tensor_scalar_mul(
                                        zn[:], ens[j][ht][:],
                                        vcol_sb[:, ht : ht + 1],
                                    )
                                else:
                                    nc.vector.scalar_tensor_tensor(
                                        zn[:], ens[j][ht][:],
                                        vcol_sb[:, ht : ht + 1], z[:],
                                        AluOpType.mult, AluOpType.add,
                                    )
                                z = zn
                            for lh in range(2):
                                nc.tensor.matmul(
                                    ps_sc[:, lh * 512 : (lh + 1) * 512],
                                    lhsT=mask16_sb[:, msl],
                                    rhs=z[:, lh * 512 : (lh + 1) * 512],
                                    start=(b == 0),
                                    stop=False,
                                )
                        else:
                            # last batch: contract v directly on the PE so
                            # the tail isn't gated on the DVE chain
                            for lh in range(2):
                                for ht in range(4):
                                    nc.tensor.matmul(
                                        ps_sc[:, lh * 512 : (lh + 1) * 512],
                                        lhsT=vmask16_sb[:, ht, msl],
                                        rhs=ens[j][ht][:, lh * 512 : (lh + 1) * 512],
                                        start=False,
                                        stop=(ht == 3),
                                    )

                # ---- softmax over all local batches; scores are bounded
                # (|s| < ~8), so skip the usual max subtraction ----
                ex = spool.tile([BL, L], F32, tag="ex", name="ex")
                sm = spool.tile([BL, 1], F32, tag="sm", name="sm")
                nc.scalar.activation(
                    ex[:], ps_sc[:, :], AF.Exp, accum_out=sm[:],
                )
                rc = spool.tile([BL, 1], F32, tag="rc", name="rc")
                nc.vector.reciprocal(rc[:], sm[:])
                oo = spool.tile([BL, L], F32, tag="oo", name="oo")
                nc.vector.tensor_scalar_mul(oo[:], ex[:], rc[:, 0:1])
                nc.sync.dma_start(out[:, :], oo[:])

    nc.compile()
    return nc


_cached_nc = None

_F8NP = ml_dtypes.float8_e4m3
_BFNP = ml_dtypes.bfloat16


def _prep_in_maps(hidden, encoder_outputs, W, b, v):
    hidden = np.ascontiguousarray(hidden, dtype=np.float32)
    W = np.ascontiguousarray(W, dtype=np.float32)
    b = np.ascontiguousarray(b, dtype=np.float32)
    v = np.ascontiguousarray(v, dtype=np.float32)
    e = np.asarray(encoder_outputs, dtype=np.float32)
    encT = e.transpose(1, 2, 0)                         # (B, D, L) view
    # fp8 part: d 0..255, DoubleRow-packed [b, p, i, l] with d = i*128 + p
    q8 = np.clip(encT[:, :K8, :] * S_E, -240, 240).astype(_F8NP)
    enc8 = np.ascontiguousarray(
        q8.reshape(B, 2, 128, L).transpose(0, 2, 1, 3)  # (B, 128, 2, L)
    )
    # bf16 part: d 256..639 (576.. zero pad), [b, p, kt, l] with d = 256+kt*128+p
    qb = np.zeros((B, NBT * 128, L), dtype=_BFNP)
    qb[:, : DE - K8] = (encT[:, K8:DE, :] * S_E).astype(_BFNP)
    encb = np.ascontiguousarray(qb.reshape(B, NBT, 128, L).transpose(0, 2, 1, 3))

    WeT = W[:, H:].T                                    # (D, H)
    w8 = np.clip(WeT[:K8] * S_W, -240, 240).astype(_F8NP)
    wet8 = np.ascontiguousarray(w8.reshape(2, 128, H).transpose(1, 0, 2))
    wb = np.zeros((NBT * 128, H), dtype=_BFNP)
    wb[: DE - K8] = (WeT[K8:DE] * S_W).astype(_BFNP)
    wbt = np.ascontiguousarray(wb.reshape(NBT, 128, H).transpose(1, 0, 2))

    wht = np.ascontiguousarray(W[:, :H].T).astype(_BFNP)  # (512, 512)
    bcol = np.ascontiguousarray(b.reshape(4, 128).T)      # (128, 4)
    vcol = np.ascontiguousarray(v.reshape(4, 128).T)
    mask16 = np.zeros((128, 31), dtype=_BFNP)
    mask16[:, 15] = 1
    vmask16 = np.zeros((128, 4, 31), dtype=_BFNP)
    vmask16[:, :, 15] = vcol.astype(_BFNP)
    in_maps = []
    for c in range(N_CORES):
        sl = slice(c * BL, (c + 1) * BL)
        in_maps.append(
            {
                "enc8": enc8[sl],
                "encb": encb[sl],
                "hid": np.ascontiguousarray(hidden[sl].T).astype(_BFNP),
                "wet8": wet8,
                "wbt": wbt,
                "wht": wht,
                "bcol": bcol,
                "vcol": vcol,
                "mask16": mask16,
                "vmask16": vmask16,
            }
        )
    return in_maps


def kernel(hidden, encoder_outputs, W, b, v):
    global _cached_nc
    if _cached_nc is None:
        _cached_nc = build(reps=1)
    in_maps = _prep_in_maps(hidden, encoder_outputs, W, b, v)
    res = bass_utils.run_bass_kernel_spmd(
        _cached_nc, in_maps, core_ids=list(range(N_CORES))
    )
    outs = np.concatenate([res.results[c]["out"] for c in range(N_CORES)], axis=0)
    return outs[:, None, :].astype(np.float32)
